# revision 10
# baseline (speedup 1.0000x reference)
"""nn_CrossAttention Trainium2 Bass kernel.

Sharding (8 cores): data-parallel over batch (4 samples x 2 cores) with
2-way Megatron tensor parallelism inside each pair: core = (sample, half).
Each half owns 8 of 16 attention heads (Wq cols / Wout rows) and 2048 of
4096 ff_inner channels (Wff1 cols / Wff2 rows); the tiny shared-head Wkv is
replicated.  Per-core partial outputs (attn@Wout_half + ff@Wff2_half) are
summed pairwise on the host, which also owns the final transpose (the
device computes the output feature-major).

Device kernel (per core, identical SPMD program):
  - LayerNorm token-major via bn_stats (gains folded into the weights on
    the host), then PE-transpose to feature-major.
  - All matmuls in fp32r (fp22 single-pass) with 512-wide moving operands.
  - Attention computed transposed (keys/queries feature-major, sim with
    context positions on partitions) so softmax sums fold into the
    attn@v matmul as a ones-column of the [v | 1] stationary operand.
  - FF1 (SwiGLU) interleaved with attention so TensorE hides ScalarE's
    exp() latency; out-projection accumulates the attention and FF paths
    into one PSUM group.
"""
import sys

if "/opt/trn_rl_repo" not in sys.path:
    sys.path.insert(0, "/opt/trn_rl_repo")

import numpy as np

import concourse.bass as bass  # noqa: F401  (bass must import before bacc)
import concourse.mybir as mybir
import concourse.tile as tile
from concourse import bacc, bass_utils

F32 = mybir.dt.float32
F32R = mybir.dt.float32r
AF = mybir.ActivationFunctionType
ALU = mybir.AluOpType

P = 128
B = 4           # batch
NTOK = 1024     # query tokens per sample
NCTX = 1024     # context tokens per sample
DIM = 1024
DH = 64         # head dim
HC = 8          # heads per core (16 total / 2-way TP)
QF = HC * DH    # 512 per-core q features
FFC = 2048      # per-core ff_inner channels
EPS = 1e-5
SCALE = DH ** -0.5

TT = NTOK // P   # 8 token tiles
KT = DIM // P    # 8 contraction tiles over dim
QC = NTOK // 512  # 2 moving-operand chunks of 512 tokens

_CACHED = {}


def _build(with_bias: bool):
    nc = bacc.Bacc("TRN2", target_bir_lowering=False, debug=False)

    x_d = nc.dram_tensor("x", [NTOK, DIM], F32, kind="ExternalInput").ap()
    c_d = nc.dram_tensor("ctx", [NCTX, DIM], F32, kind="ExternalInput").ap()
    wq_d = nc.dram_tensor("wq", [DIM, QF], F32R, kind="ExternalInput").ap()
    wkv_d = nc.dram_tensor("wkv", [DIM, 2 * DH], F32R, kind="ExternalInput").ap()
    wout_d = nc.dram_tensor("wout", [QF, DIM], F32R, kind="ExternalInput").ap()
    wff1_d = nc.dram_tensor("wff1", [DIM, 2 * FFC], F32R, kind="ExternalInput").ap()
    wff2_d = nc.dram_tensor("wff2", [FFC, DIM], F32R, kind="ExternalInput").ap()
    eye_d = nc.dram_tensor("eye", [P, P], F32, kind="ExternalInput").ap()
    eyer_d = nc.dram_tensor("eyer", [P, P], F32R, kind="ExternalInput").ap()
    ones_d = nc.dram_tensor("onesd", [P, 1], F32R, kind="ExternalInput").ap()
    if with_bias:
        bq_d = nc.dram_tensor("bq", [1, QF], F32, kind="ExternalInput").ap()
        bkv_d = nc.dram_tensor("bkv", [1, 2 * DH], F32, kind="ExternalInput").ap()
        bff1_d = nc.dram_tensor("bff1", [1, 2 * FFC], F32, kind="ExternalInput").ap()
    out_d = nc.dram_tensor("out", [DIM, NTOK], F32, kind="ExternalOutput").ap()

    # dram views tiled for lhsT streaming: [p, ktile, cols]
    wq_v = wq_d.rearrange("(ko p) c -> p ko c", p=P)
    wkv_v = wkv_d.rearrange("(ko p) c -> p ko c", p=P)
    wout_v = wout_d.rearrange("(ko p) c -> p ko c", p=P)
    wff1_v = wff1_d.rearrange("(ko p) c -> p ko c", p=P)
    wff2_v = wff2_d.rearrange("(ko p) c -> p ko c", p=P)

    with tile.TileContext(nc) as tc:
        with (
            tc.tile_pool(name="consts", bufs=1) as consts,
            tc.tile_pool(name="ln", bufs=2) as lnp,
            tc.tile_pool(name="small", bufs=2) as smallp,
            tc.tile_pool(name="small1", bufs=1) as smallp1,
            tc.tile_pool(name="resid", bufs=1) as resid,
            tc.tile_pool(name="big", bufs=1) as bigp,
            tc.tile_pool(name="wst", bufs=3) as wst,
            tc.tile_pool(name="attn", bufs=1) as attnp,
            tc.tile_pool(name="pm", bufs=3, space="PSUM") as pmp,
            tc.tile_pool(name="po", bufs=2, space="PSUM") as pop,
            tc.tile_pool(name="pf", bufs=3, space="PSUM") as pfp,
        ):
            ident = consts.tile([P, P], F32)
            nc.sync.dma_start(ident[:], eye_d[:])
            identr = consts.tile([P, P], F32R)
            nc.sync.dma_start(identr[:], eyer_d[:])
            eps_t = consts.tile([P, 1], F32)
            nc.vector.memset(eps_t[:], EPS)
            if with_bias:
                bq_t = consts.tile([P, QF // P], F32)
                nc.sync.dma_start(bq_t[:], bq_d.rearrange("o (fo p) -> p (o fo)", p=P))
                bkv_t = consts.tile([P, 1], F32)
                nc.sync.dma_start(bkv_t[:], bkv_d.rearrange("o (fo p) -> p (o fo)", p=P))
                bff1_t = consts.tile([P, (2 * FFC) // P], F32)
                nc.sync.dma_start(
                    bff1_t[:], bff1_d.rearrange("o (fo p) -> p (o fo)", p=P)
                )

            # persistent activations
            xn_F = resid.tile([P, KT, NTOK], F32R)      # normalized x, feature-major
            qT = resid.tile([P, QF // P, NTOK], F32R)   # queries, feature-major
            kv_sb = resid.tile([P, NCTX], F32R)         # rows 0:64 v, 64:128 k (feature-major)
            kdup = resid.tile([P, NCTX], F32R)          # rows 0:64 = copy of k
            v_aug = resid.tile([P, NCTX // P, DH + 1], F32R)  # [j-in-tile, jt, v|1]
            attn_outT = resid.tile([P, QF // P, NTOK], F32R)  # attention out, feature-major

            def layernorm_to_fmajor(src_dram, dst_fmajor):
                """LN over the last dim of [1024, 1024] src; write transposed."""
                for tt in range(TT):
                    xt = wst.tile([P, DIM], F32, tag="wbig", name="xt")
                    nc.sync.dma_start(xt[:], src_dram[tt * P:(tt + 1) * P, :])
                    st = lnp.tile([P, 2, nc.vector.BN_STATS_DIM], F32, tag="lnst")
                    xv = xt.rearrange("p (s f) -> p s f", s=2)
                    nc.vector.bn_stats(st[:, 0, :], xv[:, 0, :])
                    nc.vector.bn_stats(st[:, 1, :], xv[:, 1, :])
                    mv = lnp.tile([P, nc.vector.BN_AGGR_DIM], F32, tag="lnmv")
                    nc.vector.bn_aggr(mv[:], st[:])
                    # rstd = 1/sqrt(var + eps)
                    rstd = lnp.tile([P, 1], F32, tag="lnrs")
                    nc.scalar.activation(
                        out=rstd[:], in_=mv[:, 1:2], func=AF.Sqrt, bias=eps_t[:]
                    )
                    nc.vector.reciprocal(rstd[:], rstd[:])
                    nc.vector.tensor_scalar(
                        out=xt[:], in0=xt[:], scalar1=mv[:, 0:1], scalar2=rstd[:],
                        op0=ALU.subtract, op1=ALU.mult,
                    )
                    for dt_ in range(KT):
                        pt = pmp.tile([P, 512], F32, tag="pm")
                        nc.tensor.transpose(
                            pt[:, 0:P], xt[:, dt_ * P:(dt_ + 1) * P], ident[:]
                        )
                        nc.vector.tensor_copy(
                            dst_fmajor[:, dt_, tt * P:(tt + 1) * P], pt[:, 0:P]
                        )

            # ---- phase A/B: layernorms ----
            layernorm_to_fmajor(x_d, xn_F)
            cn_F = bigp.tile([P, KT, NCTX], F32R, tag="big", name="cn_F")
            layernorm_to_fmajor(c_d, cn_F)

            # ---- phase C: kv = cn @ Wkv (feature-major out) ----
            wkv_t = wst.tile([P, KT, 2 * DH], F32R, tag="wpair", name="wkv_t")
            nc.sync.dma_start(wkv_t[:], wkv_v[:])
            for jc in range(NCTX // 512):
                pkv = pmp.tile([P, 512], F32, tag="pm")
                for k in range(KT):
                    nc.tensor.matmul(
                        pkv[0:2 * DH, :], wkv_t[:, k, :],
                        cn_F[:, k, jc * 512:(jc + 1) * 512],
                        start=(k == 0), stop=(k == KT - 1),
                    )
                if with_bias:
                    nc.vector.tensor_scalar_add(
                        out=kv_sb[:, jc * 512:(jc + 1) * 512],
                        in0=pkv[0:2 * DH, :], scalar1=bkv_t[:],
                    )
                else:
                    nc.vector.tensor_copy(
                        kv_sb[:, jc * 512:(jc + 1) * 512], pkv[0:2 * DH, :]
                    )
            # k lives at partitions 64:128 (for the odd-head sim matmuls, whose
            # queries sit at partitions 64:128); duplicate it at 0:64 for the
            # even heads.
            nc.sync.dma_start(kdup[0:DH, :], kv_sb[DH:2 * DH, :])
            # v (partitions 0:64) transposed to token-major with a ones column
            for jt in range(NCTX // P):
                pv = pmp.tile([P, 512], F32R, tag="pm")
                nc.tensor.transpose(
                    pv[:, 0:DH], kv_sb[0:DH, jt * P:(jt + 1) * P],
                    identr[0:DH, 0:DH],
                )
                nc.vector.tensor_copy(v_aug[:, jt, 0:DH], pv[:, 0:DH])
            nc.sync.dma_start(
                v_aug[:, :, DH:DH + 1],
                bass.AP(tensor=ones_d.tensor, offset=0,
                        ap=[list(ones_d.ap[0]), [0, NCTX // P], list(ones_d.ap[1])]),
            )

            # ---- phase D: q = xn @ Wq (feature-major out) ----
            for ft in range(QF // P):
                wq_t = wst.tile([P, KT, P], F32R, tag="wpair", name="wq_t")
                nc.sync.dma_start(wq_t[:], wq_v[:, :, ft * P:(ft + 1) * P])
                for qc in range(QC):
                    pq = pmp.tile([P, 512], F32, tag="pm")
                    for k in range(KT):
                        nc.tensor.matmul(
                            pq[:], wq_t[:, k, :],
                            xn_F[:, k, qc * 512:(qc + 1) * 512],
                            start=(k == 0), stop=(k == KT - 1),
                        )
                    if with_bias:
                        nc.vector.tensor_scalar_add(
                            out=qT[:, ft, qc * 512:(qc + 1) * 512],
                            in0=pq[:], scalar1=bq_t[:, ft:ft + 1],
                        )
                    else:
                        nc.vector.tensor_copy(
                            qT[:, ft, qc * 512:(qc + 1) * 512], pq[:]
                        )

            # ---- phases E+F: attention (pairs of heads) interleaved with FF1 ----
            ff_sc = [None, None]  # per-qc silu(gate)*val, feature-major [128, 16, 512]

            def ff1_iter(qc, i):
                """One val/gate column pair (128 wide) of the SwiGLU FF."""
                wv_t = wst.tile([P, KT, P], F32R, tag="wpair", name="wv_t")
                nc.sync.dma_start(wv_t[:], wff1_v[:, :, i * P:(i + 1) * P])
                wg_t = wst.tile([P, KT, P], F32R, tag="wpair", name="wg_t")
                nc.sync.dma_start(wg_t[:], wff1_v[:, :, FFC + i * P:FFC + (i + 1) * P])
                pv_ = pfp.tile([P, 512], F32, tag="pf")
                pg_ = pfp.tile([P, 512], F32, tag="pf")
                for k in range(KT):
                    nc.tensor.matmul(
                        pv_[:], wv_t[:, k, :], xn_F[:, k, qc * 512:(qc + 1) * 512],
                        start=(k == 0), stop=(k == KT - 1),
                    )
                for k in range(KT):
                    nc.tensor.matmul(
                        pg_[:], wg_t[:, k, :], xn_F[:, k, qc * 512:(qc + 1) * 512],
                        start=(k == 0), stop=(k == KT - 1),
                    )
                if with_bias:
                    nc.vector.tensor_scalar_add(
                        out=pv_[:], in0=pv_[:], scalar1=bff1_t[:, i:i + 1]
                    )
                    nc.vector.tensor_scalar_add(
                        out=pg_[:], in0=pg_[:], scalar1=bff1_t[:, FFC // P + i:FFC // P + i + 1]
                    )
                sg = smallp.tile([P, 512], F32, tag="silu")
                nc.scalar.activation(out=sg[:], in_=pg_[:], func=AF.Sigmoid)
                nc.vector.tensor_tensor(sg[:], pg_[:], sg[:], ALU.mult)
                nc.vector.tensor_tensor(
                    ff_sc[qc][:, i, :], pv_[:], sg[:], ALU.mult
                )

            def attn_pair(ft, qc):
                """Heads (2ft, 2ft+1) for one 512-token chunk."""
                expT = attnp.tile([P, NCTX // P, 2, 512], F32R, tag="expT")
                qsl = [
                    qT[0:DH, ft, qc * 512:(qc + 1) * 512],
                    qT[DH:2 * DH, ft, qc * 512:(qc + 1) * 512],
                ]
                for jt in range(NCTX // P):
                    ps0 = pmp.tile([P, 512], F32, tag="pm")
                    ps1 = pmp.tile([P, 512], F32, tag="pm")
                    nc.tensor.matmul(
                        ps0[:], kdup[0:DH, jt * P:(jt + 1) * P], qsl[0],
                        start=True, stop=True,
                    )
                    nc.tensor.matmul(
                        ps1[:], kv_sb[DH:2 * DH, jt * P:(jt + 1) * P], qsl[1],
                        start=True, stop=True,
                    )
                    nc.scalar.activation(out=expT[:, jt, 0, :], in_=ps0[:], func=AF.Exp)
                    nc.scalar.activation(out=expT[:, jt, 1, :], in_=ps1[:], func=AF.Exp)
                po = [pop.tile([P, 512], F32, tag="po", name=f"po{e}") for e in range(2)]
                for jt in range(NCTX // P):
                    for e in range(2):
                        nc.tensor.matmul(
                            po[e][0:DH + 1, :], v_aug[:, jt, :], expT[:, jt, e, :],
                            start=(jt == 0), stop=(jt == NCTX // P - 1),
                        )
                for e in range(2):
                    rec = smallp1.tile([P, 512], F32, tag="rec")
                    # move the sums row (psum partition 64) to partition 0
                    nc.vector.tensor_copy(rec[DH:DH + 1, :], po[e][DH:DH + 1, :])
                    nc.sync.dma_start(rec[0:1, :], rec[DH:DH + 1, :])
                    nc.vector.reciprocal(rec[0:1, :], rec[0:1, :])
                    rb = smallp1.tile([DH, 512], F32, tag="rb")
                    nc.gpsimd.partition_broadcast(rb[:], rec[0:1, :])
                    if e == 0:
                        nc.vector.tensor_tensor(
                            attn_outT[0:DH, ft, qc * 512:(qc + 1) * 512],
                            po[e][0:DH, :], rb[:], ALU.mult,
                        )
                    else:
                        stg = smallp1.tile([DH, 512], F32R, tag="stg")
                        nc.vector.tensor_tensor(stg[:], po[e][0:DH, :], rb[:], ALU.mult)
                        nc.sync.dma_start(
                            attn_outT[DH:2 * DH, ft, qc * 512:(qc + 1) * 512], stg[:]
                        )

            ff_sc[0] = bigp.tile([P, FFC // P, 512], F32R, tag="big", name="ff_sc0")
            ff_it = iter(range(FFC // P))
            for ft in range(QF // P):
                for qc in range(QC):
                    attn_pair(ft, qc)
                    for _ in range(2):
                        i = next(ff_it, None)
                        if i is not None:
                            ff1_iter(0, i)
            for i in ff_it:
                ff1_iter(0, i)

            # ---- phase G: out = attn_outT' Wout + ff' Wff2, per 512-token chunk ----
            def out_proj(qc):
                for mt in range(DIM // P):
                    wo_t = wst.tile([P, QF // P, P], F32R, tag="wpair", name="wo_t")
                    nc.sync.dma_start(wo_t[:], wout_v[:, :, mt * P:(mt + 1) * P])
                    wf2_t = wst.tile([P, FFC // P, P], F32R, tag="wbig", name="wf2_t")
                    nc.sync.dma_start(wf2_t[:], wff2_v[:, :, mt * P:(mt + 1) * P])
                    pout = pmp.tile([P, 512], F32, tag="pm")
                    nkt = QF // P + FFC // P
                    for k in range(QF // P):
                        nc.tensor.matmul(
                            pout[:], wo_t[:, k, :],
                            attn_outT[:, k, qc * 512:(qc + 1) * 512],
                            start=(k == 0), stop=False,
                        )
                    for k in range(FFC // P):
                        nc.tensor.matmul(
                            pout[:], wf2_t[:, k, :], ff_sc[qc][:, k, :],
                            start=False, stop=(k == FFC // P - 1),
                        )
                    ot = smallp.tile([P, 512], F32, tag="ot")
                    nc.vector.tensor_copy(ot[:], pout[:])
                    nc.sync.dma_start(
                        out_d[mt * P:(mt + 1) * P, qc * 512:(qc + 1) * 512], ot[:]
                    )

            out_proj(0)
            ff_sc[1] = bigp.tile([P, FFC // P, 512], F32R, tag="big", name="ff_sc1")
            for i in range(FFC // P):
                ff1_iter(1, i)
            out_proj(1)

    nc.compile()
    return nc


def _get_program(with_bias: bool):
    key = ("nc", with_bias)
    if key not in _CACHED:
        _CACHED[key] = _build(with_bias)
    return _CACHED[key]


def kernel(x, context, ln_x_g, ln_x_b, ln_c_g, ln_c_b, Wq, Wkv, Wout, Wff1, Wff2):
    x = np.asarray(x, np.float32)
    context = np.asarray(context, np.float32)
    ln_x_g = np.asarray(ln_x_g, np.float32)
    ln_x_b = np.asarray(ln_x_b, np.float32)
    ln_c_g = np.asarray(ln_c_g, np.float32)
    ln_c_b = np.asarray(ln_c_b, np.float32)
    Wq = np.asarray(Wq, np.float32)
    Wkv = np.asarray(Wkv, np.float32)
    Wout = np.asarray(Wout, np.float32)
    Wff1 = np.asarray(Wff1, np.float32)
    Wff2 = np.asarray(Wff2, np.float32)

    # fold LN gains (and the attention scale) into the weights
    wq_eff = (ln_x_g[:, None] * Wq) * SCALE          # [1024, 1024]
    wkv_eff = ln_c_g[:, None] * Wkv                  # [1024, 128]
    # device kv layout: v at features 0:64, k at 64:128
    wkv_eff = np.concatenate([wkv_eff[:, DH:], wkv_eff[:, :DH]], axis=1)
    wff1_eff = ln_x_g[:, None] * Wff1                # [1024, 8192]
    with_bias = bool(np.any(ln_x_b != 0.0) or np.any(ln_c_b != 0.0))
    if with_bias:
        bq_eff = (ln_x_b @ Wq) * SCALE               # [1024]
        bkv_eff = ln_c_b @ Wkv                       # [128]
        bkv_eff = np.concatenate([bkv_eff[DH:], bkv_eff[:DH]])
        bff1_eff = ln_x_b @ Wff1                     # [8192]

    eye = np.eye(P, dtype=np.float32)
    onesd = np.ones((P, 1), np.float32)
    in_maps = []
    for c in range(8):
        s, t = c // 2, c % 2
        m = {
            "x": np.ascontiguousarray(x[s]),
            "ctx": np.ascontiguousarray(context[s]),
            "wq": np.ascontiguousarray(wq_eff[:, QF * t:QF * (t + 1)]),
            "wkv": np.ascontiguousarray(wkv_eff),
            "wout": np.ascontiguousarray(Wout[QF * t:QF * (t + 1), :]),
            "wff1": np.ascontiguousarray(np.concatenate(
                [wff1_eff[:, FFC * t:FFC * (t + 1)],
                 wff1_eff[:, 2 * FFC + FFC * t:2 * FFC + FFC * (t + 1)]], axis=1)),
            "wff2": np.ascontiguousarray(Wff2[FFC * t:FFC * (t + 1), :]),
            "eye": eye,
            "eyer": eye,
            "onesd": onesd,
        }
        if with_bias:
            m["bq"] = np.ascontiguousarray(bq_eff[None, QF * t:QF * (t + 1)])
            m["bkv"] = np.ascontiguousarray(bkv_eff[None, :])
            m["bff1"] = np.ascontiguousarray(np.concatenate(
                [bff1_eff[None, FFC * t:FFC * (t + 1)],
                 bff1_eff[None, 2 * FFC + FFC * t:2 * FFC + FFC * (t + 1)]], axis=1))
        in_maps.append(m)

    nc = _get_program(with_bias)
    _CACHED["in_maps"] = in_maps
    res = bass_utils.run_bass_kernel_spmd(nc, in_maps, core_ids=list(range(8)))
    out = np.empty((B, NTOK, DIM), np.float32)
    for s in range(B):
        out[s] = (res.results[2 * s]["out"] + res.results[2 * s + 1]["out"]).T
    return out


# revision 13
# speedup vs baseline: 1.1554x; 1.1554x over previous
"""nn_CrossAttention Trainium2 Bass kernel.

Sharding (8 cores): data-parallel over batch (4 samples x 2 cores) with
2-way Megatron tensor parallelism inside each pair: core = (sample, half).
Each half owns 8 of 16 attention heads (Wq cols / Wout rows) and 2048 of
4096 ff_inner channels (Wff1 cols / Wff2 rows); the tiny shared-head Wkv is
replicated.  Per-core partial outputs (attn@Wout_half + ff@Wff2_half) are
summed pairwise on the host, which also owns the final transpose (the
device computes the output feature-major).

Device kernel (per core, identical SPMD program):
  - LayerNorm token-major via bn_stats (gains folded into the weights on
    the host), then PE-transpose to feature-major.
  - All matmuls in bf16 with 512-wide moving operands (fp32 PSUM accum).
  - Attention computed transposed (keys/queries feature-major, sim with
    context positions on partitions) so softmax sums fold into the
    attn@v matmul as a ones-column of the [v | 1] stationary operand.
  - FF1 (SwiGLU) interleaved with attention so TensorE hides ScalarE's
    exp() latency; out-projection accumulates the attention and FF paths
    into one PSUM group.
"""
import sys

if "/opt/trn_rl_repo" not in sys.path:
    sys.path.insert(0, "/opt/trn_rl_repo")

import numpy as np

import concourse.bass as bass  # noqa: F401  (bass must import before bacc)
import concourse.mybir as mybir
import concourse.tile as tile
from concourse import bacc, bass_utils

F32 = mybir.dt.float32
F32R = mybir.dt.float32r
BF16 = mybir.dt.bfloat16
AF = mybir.ActivationFunctionType
ALU = mybir.AluOpType

P = 128
B = 4           # batch
NTOK = 1024     # query tokens per sample
NCTX = 1024     # context tokens per sample
DIM = 1024
DH = 64         # head dim
HC = 8          # heads per core (16 total / 2-way TP)
QF = HC * DH    # 512 per-core q features
FFC = 2048      # per-core ff_inner channels
EPS = 1e-5
SCALE = DH ** -0.5

TT = NTOK // P   # 8 token tiles
KT = DIM // P    # 8 contraction tiles over dim
QC = NTOK // 512  # 2 moving-operand chunks of 512 tokens

_CACHED = {}


def _build(with_bias: bool):
    nc = bacc.Bacc("TRN2", target_bir_lowering=False, debug=False)

    x_d = nc.dram_tensor("x", [NTOK, DIM], F32, kind="ExternalInput").ap()
    c_d = nc.dram_tensor("ctx", [NCTX, DIM], F32, kind="ExternalInput").ap()
    wq_d = nc.dram_tensor("wq", [DIM, QF], BF16, kind="ExternalInput").ap()
    wkv_d = nc.dram_tensor("wkv", [DIM, 2 * DH], BF16, kind="ExternalInput").ap()
    wout_d = nc.dram_tensor("wout", [QF, DIM], BF16, kind="ExternalInput").ap()
    wff1_d = nc.dram_tensor("wff1", [DIM, 2 * FFC], BF16, kind="ExternalInput").ap()
    wff2_d = nc.dram_tensor("wff2", [FFC, DIM], BF16, kind="ExternalInput").ap()
    eyer_d = nc.dram_tensor("eyer", [P, P], BF16, kind="ExternalInput").ap()
    ones_d = nc.dram_tensor("onesd", [P, 1], BF16, kind="ExternalInput").ap()
    if with_bias:
        bq_d = nc.dram_tensor("bq", [1, QF], F32, kind="ExternalInput").ap()
        bkv_d = nc.dram_tensor("bkv", [1, 2 * DH], F32, kind="ExternalInput").ap()
        bff1_d = nc.dram_tensor("bff1", [1, 2 * FFC], F32, kind="ExternalInput").ap()
    out_d = nc.dram_tensor("out", [DIM, NTOK], F32, kind="ExternalOutput").ap()

    # dram views tiled for lhsT streaming: [p, ktile, cols]
    wq_v = wq_d.rearrange("(ko p) c -> p ko c", p=P)
    wkv_v = wkv_d.rearrange("(ko p) c -> p ko c", p=P)
    wout_v = wout_d.rearrange("(ko p) c -> p ko c", p=P)
    wff1_v = wff1_d.rearrange("(ko p) c -> p ko c", p=P)
    wff2_v = wff2_d.rearrange("(ko p) c -> p ko c", p=P)

    with tile.TileContext(nc) as tc:
        with (
            tc.tile_pool(name="consts", bufs=1) as consts,
            tc.tile_pool(name="ln", bufs=3) as lnp,
            tc.tile_pool(name="small", bufs=2) as smallp,
            tc.tile_pool(name="small1", bufs=1) as smallp1,
            tc.tile_pool(name="resid", bufs=1) as resid,
            tc.tile_pool(name="big", bufs=1) as bigp,
            tc.tile_pool(name="wst", bufs=4) as wst,
            tc.tile_pool(name="attn", bufs=2) as attnp,
            tc.tile_pool(name="pm", bufs=3, space="PSUM") as pmp,
            tc.tile_pool(name="po", bufs=2, space="PSUM") as pop,
            tc.tile_pool(name="pf", bufs=3, space="PSUM") as pfp,
        ):
            identr = consts.tile([P, P], BF16)
            nc.sync.dma_start(identr[:], eyer_d[:])
            eps_t = consts.tile([P, 1], F32)
            nc.vector.memset(eps_t[:], EPS)
            if with_bias:
                bq_t = consts.tile([P, QF // P], F32)
                nc.sync.dma_start(bq_t[:], bq_d.rearrange("o (fo p) -> p (o fo)", p=P))
                bkv_t = consts.tile([P, 1], F32)
                nc.sync.dma_start(bkv_t[:], bkv_d.rearrange("o (fo p) -> p (o fo)", p=P))
                bff1_t = consts.tile([P, (2 * FFC) // P], F32)
                nc.sync.dma_start(
                    bff1_t[:], bff1_d.rearrange("o (fo p) -> p (o fo)", p=P)
                )

            # persistent activations
            xn_F = resid.tile([P, KT, NTOK], BF16)      # normalized x, feature-major
            qT = resid.tile([P, QF // P, NTOK], BF16)   # queries, feature-major
            kv_sb = resid.tile([P, NCTX], BF16)         # rows 0:64 v, 64:128 k (feature-major)
            kdup = resid.tile([P, NCTX], BF16)          # rows 0:64 = copy of k
            v_aug = resid.tile([P, NCTX // P, DH + 1], BF16)  # [j-in-tile, jt, v|1]
            attn_outT = resid.tile([P, QF // P, NTOK], BF16)  # attention out, feature-major

            def layernorm_to_fmajor(src_dram, dst_fmajor):
                """LN over the last dim of [1024, 1024] src; write transposed."""
                for tt in range(TT):
                    xt = wst.tile([P, DIM], F32, tag="wbig", name="xt")
                    nc.sync.dma_start(xt[:], src_dram[tt * P:(tt + 1) * P, :])
                    st = lnp.tile([P, 2, nc.vector.BN_STATS_DIM], F32, tag="lnst")
                    xv = xt.rearrange("p (s f) -> p s f", s=2)
                    nc.vector.bn_stats(st[:, 0, :], xv[:, 0, :])
                    nc.vector.bn_stats(st[:, 1, :], xv[:, 1, :])
                    mv = lnp.tile([P, nc.vector.BN_AGGR_DIM], F32, tag="lnmv")
                    nc.vector.bn_aggr(mv[:], st[:])
                    # rstd = 1/sqrt(var + eps)
                    rstd = lnp.tile([P, 1], F32, tag="lnrs")
                    nc.scalar.activation(
                        out=rstd[:], in_=mv[:, 1:2], func=AF.Sqrt, bias=eps_t[:]
                    )
                    nc.vector.reciprocal(rstd[:], rstd[:])
                    xh = lnp.tile([P, DIM], BF16, tag="lnh")
                    nc.vector.tensor_scalar(
                        out=xh[:], in0=xt[:], scalar1=mv[:, 0:1], scalar2=rstd[:],
                        op0=ALU.subtract, op1=ALU.mult,
                    )
                    for dt_ in range(KT):
                        pt = pmp.tile([P, 512], BF16, tag="pm", name="pt")
                        nc.tensor.transpose(
                            pt[:, 0:P], xh[:, dt_ * P:(dt_ + 1) * P], identr[:]
                        )
                        if dt_ % 2 == 0:
                            nc.vector.tensor_copy(
                                dst_fmajor[:, dt_, tt * P:(tt + 1) * P], pt[:, 0:P]
                            )
                        else:
                            nc.scalar.activation(
                                out=dst_fmajor[:, dt_, tt * P:(tt + 1) * P],
                                in_=pt[:, 0:P], func=AF.Copy,
                            )

            # ---- phase A/B: layernorms ----
            layernorm_to_fmajor(x_d, xn_F)
            cn_F = bigp.tile([P, KT, NCTX], BF16, tag="big", name="cn_F")
            layernorm_to_fmajor(c_d, cn_F)

            # ---- phase C: kv = cn @ Wkv (feature-major out) ----
            wkv_t = wst.tile([P, KT, 2 * DH], BF16, tag="wpair", name="wkv_t")
            nc.sync.dma_start(wkv_t[:], wkv_v[:])
            for jc in range(NCTX // 512):
                pkv = pmp.tile([P, 512], F32, tag="pm")
                for k in range(KT):
                    nc.tensor.matmul(
                        pkv[0:2 * DH, :], wkv_t[:, k, :],
                        cn_F[:, k, jc * 512:(jc + 1) * 512],
                        start=(k == 0), stop=(k == KT - 1),
                    )
                if with_bias:
                    nc.vector.tensor_scalar_add(
                        out=kv_sb[:, jc * 512:(jc + 1) * 512],
                        in0=pkv[0:2 * DH, :], scalar1=bkv_t[:],
                    )
                else:
                    nc.vector.tensor_copy(
                        kv_sb[:, jc * 512:(jc + 1) * 512], pkv[0:2 * DH, :]
                    )
            # k lives at partitions 64:128 (for the odd-head sim matmuls, whose
            # queries sit at partitions 64:128); duplicate it at 0:64 for the
            # even heads.
            nc.sync.dma_start(kdup[0:DH, :], kv_sb[DH:2 * DH, :])
            # v (partitions 0:64) transposed to token-major with a ones column
            for jt in range(NCTX // P):
                pv = pmp.tile([P, 512], BF16, tag="pm")
                nc.tensor.transpose(
                    pv[:, 0:DH], kv_sb[0:DH, jt * P:(jt + 1) * P],
                    identr[0:DH, 0:DH],
                )
                nc.vector.tensor_copy(v_aug[:, jt, 0:DH], pv[:, 0:DH])
            nc.sync.dma_start(
                v_aug[:, :, DH:DH + 1],
                bass.AP(tensor=ones_d.tensor, offset=0,
                        ap=[list(ones_d.ap[0]), [0, NCTX // P], list(ones_d.ap[1])]),
            )

            # ---- phase D: q = xn @ Wq (feature-major out) ----
            for ft in range(QF // P):
                wq_t = wst.tile([P, KT, P], BF16, tag="wpair", name="wq_t")
                nc.sync.dma_start(wq_t[:], wq_v[:, :, ft * P:(ft + 1) * P])
                for qc in range(QC):
                    pq = pmp.tile([P, 512], F32, tag="pm")
                    for k in range(KT):
                        nc.tensor.matmul(
                            pq[:], wq_t[:, k, :],
                            xn_F[:, k, qc * 512:(qc + 1) * 512],
                            start=(k == 0), stop=(k == KT - 1),
                        )
                    if with_bias:
                        nc.vector.tensor_scalar_add(
                            out=qT[:, ft, qc * 512:(qc + 1) * 512],
                            in0=pq[:], scalar1=bq_t[:, ft:ft + 1],
                        )
                    else:
                        nc.vector.tensor_copy(
                            qT[:, ft, qc * 512:(qc + 1) * 512], pq[:]
                        )

            # ---- phases E+F: attention (pairs of heads) interleaved with FF1 ----
            ff_sc = [None, None]  # per-qc silu(gate)*val, feature-major [128, 16, 512]

            def ff1_iter(qc, i):
                """One val/gate column pair (128 wide) of the SwiGLU FF."""
                wv_t = wst.tile([P, KT, P], BF16, tag="wpair", name="wv_t")
                nc.sync.dma_start(wv_t[:], wff1_v[:, :, i * P:(i + 1) * P])
                wg_t = wst.tile([P, KT, P], BF16, tag="wpair", name="wg_t")
                nc.sync.dma_start(wg_t[:], wff1_v[:, :, FFC + i * P:FFC + (i + 1) * P])
                pv_ = pfp.tile([P, 512], F32, tag="pf")
                pg_ = pfp.tile([P, 512], F32, tag="pf")
                for k in range(KT):
                    nc.tensor.matmul(
                        pv_[:], wv_t[:, k, :], xn_F[:, k, qc * 512:(qc + 1) * 512],
                        start=(k == 0), stop=(k == KT - 1),
                    )
                for k in range(KT):
                    nc.tensor.matmul(
                        pg_[:], wg_t[:, k, :], xn_F[:, k, qc * 512:(qc + 1) * 512],
                        start=(k == 0), stop=(k == KT - 1),
                    )
                if with_bias:
                    nc.vector.tensor_scalar_add(
                        out=pv_[:], in0=pv_[:], scalar1=bff1_t[:, i:i + 1]
                    )
                    nc.vector.tensor_scalar_add(
                        out=pg_[:], in0=pg_[:], scalar1=bff1_t[:, FFC // P + i:FFC // P + i + 1]
                    )
                sg = smallp.tile([P, 512], F32, tag="silu")
                nc.scalar.activation(out=sg[:], in_=pg_[:], func=AF.Sigmoid)
                nc.vector.tensor_tensor(sg[:], pg_[:], sg[:], ALU.mult)
                nc.vector.tensor_tensor(
                    ff_sc[qc][:, i, :], pv_[:], sg[:], ALU.mult
                )

            def attn_pair(ft, qc):
                """Heads (2ft, 2ft+1) for one 512-token chunk."""
                expT = attnp.tile([P, NCTX // P, 2, 512], BF16, tag="expT")
                qsl = [
                    qT[0:DH, ft, qc * 512:(qc + 1) * 512],
                    qT[DH:2 * DH, ft, qc * 512:(qc + 1) * 512],
                ]
                for jt in range(NCTX // P):
                    ps0 = pmp.tile([P, 512], F32, tag="pm")
                    ps1 = pmp.tile([P, 512], F32, tag="pm")
                    nc.tensor.matmul(
                        ps0[:], kdup[0:DH, jt * P:(jt + 1) * P], qsl[0],
                        start=True, stop=True,
                    )
                    nc.tensor.matmul(
                        ps1[:], kv_sb[DH:2 * DH, jt * P:(jt + 1) * P], qsl[1],
                        start=True, stop=True,
                    )
                    nc.scalar.activation(out=expT[:, jt, 0, :], in_=ps0[:], func=AF.Exp)
                    nc.scalar.activation(out=expT[:, jt, 1, :], in_=ps1[:], func=AF.Exp)
                po = [pop.tile([P, 512], F32, tag="po", name=f"po{e}") for e in range(2)]
                for jt in range(NCTX // P):
                    for e in range(2):
                        nc.tensor.matmul(
                            po[e][0:DH + 1, :], v_aug[:, jt, :], expT[:, jt, e, :],
                            start=(jt == 0), stop=(jt == NCTX // P - 1),
                        )
                for e in range(2):
                    rec = smallp1.tile([P, 512], F32, tag="rec")
                    # move the sums row (psum partition 64) to partition 0
                    nc.vector.tensor_copy(rec[DH:DH + 1, :], po[e][DH:DH + 1, :])
                    nc.sync.dma_start(rec[0:1, :], rec[DH:DH + 1, :])
                    nc.vector.reciprocal(rec[0:1, :], rec[0:1, :])
                    rb = smallp1.tile([DH, 512], F32, tag="rb")
                    nc.gpsimd.partition_broadcast(rb[:], rec[0:1, :])
                    if e == 0:
                        nc.vector.tensor_tensor(
                            attn_outT[0:DH, ft, qc * 512:(qc + 1) * 512],
                            po[e][0:DH, :], rb[:], ALU.mult,
                        )
                    else:
                        stg = smallp1.tile([DH, 512], BF16, tag="stg")
                        nc.vector.tensor_tensor(stg[:], po[e][0:DH, :], rb[:], ALU.mult)
                        nc.sync.dma_start(
                            attn_outT[DH:2 * DH, ft, qc * 512:(qc + 1) * 512], stg[:]
                        )

            ff_sc[0] = bigp.tile([P, FFC // P, 512], BF16, tag="big", name="ff_sc0")
            ff_it = iter(range(FFC // P))
            for ft in range(QF // P):
                for qc in range(QC):
                    attn_pair(ft, qc)
                    for _ in range(2):
                        i = next(ff_it, None)
                        if i is not None:
                            ff1_iter(0, i)
            for i in ff_it:
                ff1_iter(0, i)

            # ---- phase G: out = attn_outT' Wout + ff' Wff2, per 512-token chunk ----
            def out_proj(qc):
                for mt in range(DIM // P):
                    wo_t = wst.tile([P, QF // P, P], BF16, tag="wpair", name="wo_t")
                    nc.sync.dma_start(wo_t[:], wout_v[:, :, mt * P:(mt + 1) * P])
                    wf2_t = wst.tile([P, FFC // P, P], BF16, tag="wbig", name="wf2_t")
                    nc.sync.dma_start(wf2_t[:], wff2_v[:, :, mt * P:(mt + 1) * P])
                    pout = pmp.tile([P, 512], F32, tag="pm")
                    nkt = QF // P + FFC // P
                    for k in range(QF // P):
                        nc.tensor.matmul(
                            pout[:], wo_t[:, k, :],
                            attn_outT[:, k, qc * 512:(qc + 1) * 512],
                            start=(k == 0), stop=False,
                        )
                    for k in range(FFC // P):
                        nc.tensor.matmul(
                            pout[:], wf2_t[:, k, :], ff_sc[qc][:, k, :],
                            start=False, stop=(k == FFC // P - 1),
                        )
                    ot = smallp.tile([P, 512], F32, tag="ot")
                    nc.vector.tensor_copy(ot[:], pout[:])
                    nc.sync.dma_start(
                        out_d[mt * P:(mt + 1) * P, qc * 512:(qc + 1) * 512], ot[:]
                    )

            out_proj(0)
            ff_sc[1] = bigp.tile([P, FFC // P, 512], BF16, tag="big", name="ff_sc1")
            for i in range(FFC // P):
                ff1_iter(1, i)
            out_proj(1)

    nc.compile()
    return nc


def _get_program(with_bias: bool):
    key = ("nc", with_bias)
    if key not in _CACHED:
        _CACHED[key] = _build(with_bias)
    return _CACHED[key]


def kernel(x, context, ln_x_g, ln_x_b, ln_c_g, ln_c_b, Wq, Wkv, Wout, Wff1, Wff2):
    x = np.asarray(x, np.float32)
    context = np.asarray(context, np.float32)
    ln_x_g = np.asarray(ln_x_g, np.float32)
    ln_x_b = np.asarray(ln_x_b, np.float32)
    ln_c_g = np.asarray(ln_c_g, np.float32)
    ln_c_b = np.asarray(ln_c_b, np.float32)
    Wq = np.asarray(Wq, np.float32)
    Wkv = np.asarray(Wkv, np.float32)
    Wout = np.asarray(Wout, np.float32)
    Wff1 = np.asarray(Wff1, np.float32)
    Wff2 = np.asarray(Wff2, np.float32)

    # fold LN gains (and the attention scale) into the weights
    wq_eff = (ln_x_g[:, None] * Wq) * SCALE          # [1024, 1024]
    wkv_eff = ln_c_g[:, None] * Wkv                  # [1024, 128]
    # device kv layout: v at features 0:64, k at 64:128
    wkv_eff = np.concatenate([wkv_eff[:, DH:], wkv_eff[:, :DH]], axis=1)
    wff1_eff = ln_x_g[:, None] * Wff1                # [1024, 8192]
    with_bias = bool(np.any(ln_x_b != 0.0) or np.any(ln_c_b != 0.0))
    if with_bias:
        bq_eff = (ln_x_b @ Wq) * SCALE               # [1024]
        bkv_eff = ln_c_b @ Wkv                       # [128]
        bkv_eff = np.concatenate([bkv_eff[DH:], bkv_eff[:DH]])
        bff1_eff = ln_x_b @ Wff1                     # [8192]

    import ml_dtypes
    bf16 = ml_dtypes.bfloat16
    eye = np.eye(P, dtype=bf16)
    onesd = np.ones((P, 1), bf16)
    in_maps = []
    for c in range(8):
        s, t = c // 2, c % 2
        m = {
            "x": np.ascontiguousarray(x[s]),
            "ctx": np.ascontiguousarray(context[s]),
            "wq": np.ascontiguousarray(wq_eff[:, QF * t:QF * (t + 1)].astype(bf16)),
            "wkv": np.ascontiguousarray(wkv_eff.astype(bf16)),
            "wout": np.ascontiguousarray(Wout[QF * t:QF * (t + 1), :].astype(bf16)),
            "wff1": np.ascontiguousarray(np.concatenate(
                [wff1_eff[:, FFC * t:FFC * (t + 1)],
                 wff1_eff[:, 2 * FFC + FFC * t:2 * FFC + FFC * (t + 1)]],
                axis=1).astype(bf16)),
            "wff2": np.ascontiguousarray(Wff2[FFC * t:FFC * (t + 1), :].astype(bf16)),
            "eyer": eye,
            "onesd": onesd,
        }
        if with_bias:
            m["bq"] = np.ascontiguousarray(bq_eff[None, QF * t:QF * (t + 1)])
            m["bkv"] = np.ascontiguousarray(bkv_eff[None, :])
            m["bff1"] = np.ascontiguousarray(np.concatenate(
                [bff1_eff[None, FFC * t:FFC * (t + 1)],
                 bff1_eff[None, 2 * FFC + FFC * t:2 * FFC + FFC * (t + 1)]], axis=1))
        in_maps.append(m)

    nc = _get_program(with_bias)
    _CACHED["in_maps"] = in_maps
    res = bass_utils.run_bass_kernel_spmd(nc, in_maps, core_ids=list(range(8)))
    out = np.empty((B, NTOK, DIM), np.float32)
    for s in range(B):
        out[s] = (res.results[2 * s]["out"] + res.results[2 * s + 1]["out"]).T
    return out


# revision 14
# speedup vs baseline: 1.1894x; 1.0295x over previous
"""nn_CrossAttention Trainium2 Bass kernel.

Sharding (8 cores): data-parallel over batch (4 samples x 2 cores) with
2-way Megatron tensor parallelism inside each pair: core = (sample, half).
Each half owns 8 of 16 attention heads (Wq cols / Wout rows) and 2048 of
4096 ff_inner channels (Wff1 cols / Wff2 rows); the tiny shared-head Wkv is
replicated.  Per-core partial outputs (attn@Wout_half + ff@Wff2_half) are
summed pairwise on the host, which also owns the final transpose (the
device computes the output feature-major).

Device kernel (per core, identical SPMD program):
  - LayerNorm token-major via bn_stats (gains folded into the weights on
    the host), then PE-transpose to feature-major.
  - All matmuls in bf16 with 512-wide moving operands (fp32 PSUM accum).
  - Attention computed transposed (keys/queries feature-major, sim with
    context positions on partitions) so softmax sums fold into the
    attn@v matmul as a ones-column of the [v | 1] stationary operand.
  - FF1 (SwiGLU) interleaved with attention so TensorE hides ScalarE's
    exp() latency; out-projection accumulates the attention and FF paths
    into one PSUM group.
"""
import sys

if "/opt/trn_rl_repo" not in sys.path:
    sys.path.insert(0, "/opt/trn_rl_repo")

import numpy as np

import concourse.bass as bass  # noqa: F401  (bass must import before bacc)
import concourse.mybir as mybir
import concourse.tile as tile
from concourse import bacc, bass_utils

F32 = mybir.dt.float32
F32R = mybir.dt.float32r
BF16 = mybir.dt.bfloat16
AF = mybir.ActivationFunctionType
ALU = mybir.AluOpType

P = 128
B = 4           # batch
NTOK = 1024     # query tokens per sample
NCTX = 1024     # context tokens per sample
DIM = 1024
DH = 64         # head dim
HC = 8          # heads per core (16 total / 2-way TP)
QF = HC * DH    # 512 per-core q features
FFC = 2048      # per-core ff_inner channels
EPS = 1e-5
SCALE = DH ** -0.5

TT = NTOK // P   # 8 token tiles
KT = DIM // P    # 8 contraction tiles over dim
QC = NTOK // 512  # 2 moving-operand chunks of 512 tokens

_CACHED = {}


def _build(with_bias: bool):
    nc = bacc.Bacc("TRN2", target_bir_lowering=False, debug=False)

    x_d = nc.dram_tensor("x", [NTOK, DIM], F32, kind="ExternalInput").ap()
    c_d = nc.dram_tensor("ctx", [NCTX, DIM], F32, kind="ExternalInput").ap()
    wq_d = nc.dram_tensor("wq", [DIM, QF], BF16, kind="ExternalInput").ap()
    wkv_d = nc.dram_tensor("wkv", [DIM, 2 * DH], BF16, kind="ExternalInput").ap()
    wout_d = nc.dram_tensor("wout", [QF, DIM], BF16, kind="ExternalInput").ap()
    wff1_d = nc.dram_tensor("wff1", [DIM, 2 * FFC], BF16, kind="ExternalInput").ap()
    wff2_d = nc.dram_tensor("wff2", [FFC, DIM], BF16, kind="ExternalInput").ap()
    eyer_d = nc.dram_tensor("eyer", [P, P], BF16, kind="ExternalInput").ap()
    ones_d = nc.dram_tensor("onesd", [P, 1], BF16, kind="ExternalInput").ap()
    if with_bias:
        bq_d = nc.dram_tensor("bq", [1, QF], F32, kind="ExternalInput").ap()
        bkv_d = nc.dram_tensor("bkv", [1, 2 * DH], F32, kind="ExternalInput").ap()
        bff1_d = nc.dram_tensor("bff1", [1, 2 * FFC], F32, kind="ExternalInput").ap()
    out_d = nc.dram_tensor("out", [DIM, NTOK], F32, kind="ExternalOutput").ap()

    # dram views tiled for lhsT streaming: [p, ktile, cols]
    wq_v = wq_d.rearrange("(ko p) c -> p ko c", p=P)
    wkv_v = wkv_d.rearrange("(ko p) c -> p ko c", p=P)
    wout_v = wout_d.rearrange("(ko p) c -> p ko c", p=P)
    wff1_v = wff1_d.rearrange("(ko p) c -> p ko c", p=P)
    wff2_v = wff2_d.rearrange("(ko p) c -> p ko c", p=P)

    with tile.TileContext(nc) as tc:
        with (
            tc.tile_pool(name="consts", bufs=1) as consts,
            tc.tile_pool(name="ln", bufs=3) as lnp,
            tc.tile_pool(name="small", bufs=2) as smallp,
            tc.tile_pool(name="small1", bufs=1) as smallp1,
            tc.tile_pool(name="resid", bufs=1) as resid,
            tc.tile_pool(name="big", bufs=1) as bigp,
            tc.tile_pool(name="wst", bufs=4) as wst,
            tc.tile_pool(name="attn", bufs=2) as attnp,
            tc.tile_pool(name="pm", bufs=3, space="PSUM") as pmp,
            tc.tile_pool(name="po", bufs=2, space="PSUM") as pop,
            tc.tile_pool(name="pf", bufs=3, space="PSUM") as pfp,
        ):
            identr = consts.tile([P, P], BF16)
            nc.sync.dma_start(identr[:], eyer_d[:])
            eps_t = consts.tile([P, 1], F32)
            nc.vector.memset(eps_t[:], EPS)
            if with_bias:
                bq_t = consts.tile([P, QF // P], F32)
                nc.sync.dma_start(bq_t[:], bq_d.rearrange("o (fo p) -> p (o fo)", p=P))
                bkv_t = consts.tile([P, 1], F32)
                nc.sync.dma_start(bkv_t[:], bkv_d.rearrange("o (fo p) -> p (o fo)", p=P))
                bff1_t = consts.tile([P, (2 * FFC) // P], F32)
                nc.sync.dma_start(
                    bff1_t[:], bff1_d.rearrange("o (fo p) -> p (o fo)", p=P)
                )

            # persistent activations
            xn_F = resid.tile([P, KT, NTOK], BF16)      # normalized x, feature-major
            qT = resid.tile([P, QF // P, NTOK], BF16)   # queries, feature-major
            kv_sb = resid.tile([P, NCTX], BF16)         # rows 0:64 v, 64:128 k (feature-major)
            kdup = resid.tile([P, NCTX], BF16)          # rows 0:64 = copy of k
            v_aug = resid.tile([P, NCTX // P, DH + 1], BF16)  # [j-in-tile, jt, v|1]
            attn_outT = resid.tile([P, QF // P, NTOK], BF16)  # attention out, feature-major

            def layernorm_iter(src_dram, dst_fmajor, tt):
                if True:
                    xt = wst.tile([P, DIM], F32, tag="wbig", name="xt")
                    nc.sync.dma_start(xt[:], src_dram[tt * P:(tt + 1) * P, :])
                    st = lnp.tile([P, 2, nc.vector.BN_STATS_DIM], F32, tag="lnst")
                    xv = xt.rearrange("p (s f) -> p s f", s=2)
                    nc.vector.bn_stats(st[:, 0, :], xv[:, 0, :])
                    nc.vector.bn_stats(st[:, 1, :], xv[:, 1, :])
                    mv = lnp.tile([P, nc.vector.BN_AGGR_DIM], F32, tag="lnmv")
                    nc.vector.bn_aggr(mv[:], st[:])
                    # rstd = 1/sqrt(var + eps)
                    rstd = lnp.tile([P, 1], F32, tag="lnrs")
                    nc.scalar.activation(
                        out=rstd[:], in_=mv[:, 1:2], func=AF.Sqrt, bias=eps_t[:]
                    )
                    nc.vector.reciprocal(rstd[:], rstd[:])
                    xh = lnp.tile([P, DIM], BF16, tag="lnh")
                    nc.vector.tensor_scalar(
                        out=xh[:], in0=xt[:], scalar1=mv[:, 0:1], scalar2=rstd[:],
                        op0=ALU.subtract, op1=ALU.mult,
                    )
                    for dt_ in range(KT):
                        pt = pmp.tile([P, 512], BF16, tag="pm", name="pt")
                        nc.tensor.transpose(
                            pt[:, 0:P], xh[:, dt_ * P:(dt_ + 1) * P], identr[:]
                        )
                        if dt_ % 2 == 0:
                            nc.vector.tensor_copy(
                                dst_fmajor[:, dt_, tt * P:(tt + 1) * P], pt[:, 0:P]
                            )
                        else:
                            nc.scalar.activation(
                                out=dst_fmajor[:, dt_, tt * P:(tt + 1) * P],
                                in_=pt[:, 0:P], func=AF.Copy,
                            )

            # ---- phase A: layernorm(x) ----
            for tt in range(TT):
                layernorm_iter(x_d, xn_F, tt)
            cn_F = bigp.tile([P, KT, NCTX], BF16, tag="bigc", name="cn_F")

            # ---- phase D: q = xn @ Wq, interleaved with layernorm(ctx) ----
            for ft in range(QF // P):
                layernorm_iter(c_d, cn_F, 2 * ft)
                layernorm_iter(c_d, cn_F, 2 * ft + 1)
                wq_t = wst.tile([P, KT, P], BF16, tag="wpair", name="wq_t")
                nc.sync.dma_start(wq_t[:], wq_v[:, :, ft * P:(ft + 1) * P])
                for qc in range(QC):
                    pq = pmp.tile([P, 512], F32, tag="pm")
                    for k in range(KT):
                        nc.tensor.matmul(
                            pq[:], wq_t[:, k, :],
                            xn_F[:, k, qc * 512:(qc + 1) * 512],
                            start=(k == 0), stop=(k == KT - 1),
                        )
                    if with_bias:
                        nc.vector.tensor_scalar_add(
                            out=qT[:, ft, qc * 512:(qc + 1) * 512],
                            in0=pq[:], scalar1=bq_t[:, ft:ft + 1],
                        )
                    else:
                        nc.vector.tensor_copy(
                            qT[:, ft, qc * 512:(qc + 1) * 512], pq[:]
                        )

            # ---- phase C: kv = cn @ Wkv (feature-major out) ----
            wkv_t = wst.tile([P, KT, 2 * DH], BF16, tag="wpair", name="wkv_t")
            nc.sync.dma_start(wkv_t[:], wkv_v[:])
            for jc in range(NCTX // 512):
                pkv = pmp.tile([P, 512], F32, tag="pm")
                for k in range(KT):
                    nc.tensor.matmul(
                        pkv[0:2 * DH, :], wkv_t[:, k, :],
                        cn_F[:, k, jc * 512:(jc + 1) * 512],
                        start=(k == 0), stop=(k == KT - 1),
                    )
                if with_bias:
                    nc.vector.tensor_scalar_add(
                        out=kv_sb[:, jc * 512:(jc + 1) * 512],
                        in0=pkv[0:2 * DH, :], scalar1=bkv_t[:],
                    )
                else:
                    nc.vector.tensor_copy(
                        kv_sb[:, jc * 512:(jc + 1) * 512], pkv[0:2 * DH, :]
                    )
            # k lives at partitions 64:128 (for the odd-head sim matmuls, whose
            # queries sit at partitions 64:128); duplicate it at 0:64 for the
            # even heads.
            nc.sync.dma_start(kdup[0:DH, :], kv_sb[DH:2 * DH, :])
            # v (partitions 0:64) transposed to token-major with a ones column
            for jt in range(NCTX // P):
                pv = pmp.tile([P, 512], BF16, tag="pm")
                nc.tensor.transpose(
                    pv[:, 0:DH], kv_sb[0:DH, jt * P:(jt + 1) * P],
                    identr[0:DH, 0:DH],
                )
                nc.vector.tensor_copy(v_aug[:, jt, 0:DH], pv[:, 0:DH])
            nc.sync.dma_start(
                v_aug[:, :, DH:DH + 1],
                bass.AP(tensor=ones_d.tensor, offset=0,
                        ap=[list(ones_d.ap[0]), [0, NCTX // P], list(ones_d.ap[1])]),
            )

            # ---- phases E+F: attention (pairs of heads) interleaved with FF1 ----
            ff_sc = [None, None]  # per-qc silu(gate)*val, feature-major [128, 16, 512]

            def ff1_iter(qc, i):
                """One val/gate column pair (128 wide) of the SwiGLU FF."""
                wv_t = wst.tile([P, KT, P], BF16, tag="wpair", name="wv_t")
                nc.sync.dma_start(wv_t[:], wff1_v[:, :, i * P:(i + 1) * P])
                wg_t = wst.tile([P, KT, P], BF16, tag="wpair", name="wg_t")
                nc.sync.dma_start(wg_t[:], wff1_v[:, :, FFC + i * P:FFC + (i + 1) * P])
                pv_ = pfp.tile([P, 512], F32, tag="pf")
                pg_ = pfp.tile([P, 512], F32, tag="pf")
                for k in range(KT):
                    nc.tensor.matmul(
                        pv_[:], wv_t[:, k, :], xn_F[:, k, qc * 512:(qc + 1) * 512],
                        start=(k == 0), stop=(k == KT - 1),
                    )
                for k in range(KT):
                    nc.tensor.matmul(
                        pg_[:], wg_t[:, k, :], xn_F[:, k, qc * 512:(qc + 1) * 512],
                        start=(k == 0), stop=(k == KT - 1),
                    )
                if with_bias:
                    nc.vector.tensor_scalar_add(
                        out=pv_[:], in0=pv_[:], scalar1=bff1_t[:, i:i + 1]
                    )
                    nc.vector.tensor_scalar_add(
                        out=pg_[:], in0=pg_[:], scalar1=bff1_t[:, FFC // P + i:FFC // P + i + 1]
                    )
                sg = smallp.tile([P, 512], F32, tag="silu")
                nc.scalar.activation(out=sg[:], in_=pg_[:], func=AF.Sigmoid)
                nc.vector.tensor_tensor(sg[:], pg_[:], sg[:], ALU.mult)
                nc.vector.tensor_tensor(
                    ff_sc[qc][:, i, :], pv_[:], sg[:], ALU.mult
                )

            def attn_pair(ft, qc):
                """Heads (2ft, 2ft+1) for one 512-token chunk."""
                expT = attnp.tile([P, NCTX // P, 2, 512], BF16, tag="expT")
                qsl = [
                    qT[0:DH, ft, qc * 512:(qc + 1) * 512],
                    qT[DH:2 * DH, ft, qc * 512:(qc + 1) * 512],
                ]
                for jt in range(NCTX // P):
                    ps0 = pmp.tile([P, 512], F32, tag="pm")
                    ps1 = pmp.tile([P, 512], F32, tag="pm")
                    nc.tensor.matmul(
                        ps0[:], kdup[0:DH, jt * P:(jt + 1) * P], qsl[0],
                        start=True, stop=True,
                    )
                    nc.tensor.matmul(
                        ps1[:], kv_sb[DH:2 * DH, jt * P:(jt + 1) * P], qsl[1],
                        start=True, stop=True,
                    )
                    nc.scalar.activation(out=expT[:, jt, 0, :], in_=ps0[:], func=AF.Exp)
                    nc.scalar.activation(out=expT[:, jt, 1, :], in_=ps1[:], func=AF.Exp)
                po = [pop.tile([P, 512], F32, tag="po", name=f"po{e}") for e in range(2)]
                for jt in range(NCTX // P):
                    for e in range(2):
                        nc.tensor.matmul(
                            po[e][0:DH + 1, :], v_aug[:, jt, :], expT[:, jt, e, :],
                            start=(jt == 0), stop=(jt == NCTX // P - 1),
                        )
                for e in range(2):
                    rec = smallp1.tile([P, 512], F32, tag="rec")
                    # move the sums row (psum partition 64) to partition 0
                    nc.vector.tensor_copy(rec[DH:DH + 1, :], po[e][DH:DH + 1, :])
                    nc.sync.dma_start(rec[0:1, :], rec[DH:DH + 1, :])
                    nc.vector.reciprocal(rec[0:1, :], rec[0:1, :])
                    rb = smallp1.tile([DH, 512], F32, tag="rb")
                    nc.gpsimd.partition_broadcast(rb[:], rec[0:1, :])
                    if e == 0:
                        nc.vector.tensor_tensor(
                            attn_outT[0:DH, ft, qc * 512:(qc + 1) * 512],
                            po[e][0:DH, :], rb[:], ALU.mult,
                        )
                    else:
                        stg = smallp1.tile([DH, 512], BF16, tag="stg")
                        nc.vector.tensor_tensor(stg[:], po[e][0:DH, :], rb[:], ALU.mult)
                        nc.sync.dma_start(
                            attn_outT[DH:2 * DH, ft, qc * 512:(qc + 1) * 512], stg[:]
                        )

            ff_sc[0] = bigp.tile([P, FFC // P, 512], BF16, tag="big", name="ff_sc0")
            ff_it = iter(range(FFC // P))
            for ft in range(QF // P):
                for qc in range(QC):
                    attn_pair(ft, qc)
                    for _ in range(2):
                        i = next(ff_it, None)
                        if i is not None:
                            ff1_iter(0, i)
            for i in ff_it:
                ff1_iter(0, i)

            # ---- phase G: out = attn_outT' Wout + ff' Wff2, per 512-token chunk ----
            def out_proj(qc, extra=None):
                for mt in range(DIM // P):
                    if extra is not None:
                        extra(mt)
                    wo_t = wst.tile([P, QF // P, P], BF16, tag="wpair", name="wo_t")
                    nc.sync.dma_start(wo_t[:], wout_v[:, :, mt * P:(mt + 1) * P])
                    wf2_t = wst.tile([P, FFC // P, P], BF16, tag="wbig", name="wf2_t")
                    nc.sync.dma_start(wf2_t[:], wff2_v[:, :, mt * P:(mt + 1) * P])
                    pout = pmp.tile([P, 512], F32, tag="pm")
                    nkt = QF // P + FFC // P
                    for k in range(QF // P):
                        nc.tensor.matmul(
                            pout[:], wo_t[:, k, :],
                            attn_outT[:, k, qc * 512:(qc + 1) * 512],
                            start=(k == 0), stop=False,
                        )
                    for k in range(FFC // P):
                        nc.tensor.matmul(
                            pout[:], wf2_t[:, k, :], ff_sc[qc][:, k, :],
                            start=False, stop=(k == FFC // P - 1),
                        )
                    ot = smallp.tile([P, 512], F32, tag="ot")
                    nc.vector.tensor_copy(ot[:], pout[:])
                    nc.sync.dma_start(
                        out_d[mt * P:(mt + 1) * P, qc * 512:(qc + 1) * 512], ot[:]
                    )

            ff_sc[1] = bigp.tile([P, FFC // P, 512], BF16, tag="big2", name="ff_sc1")

            def _ff1_qc1(mt):
                for i in (2 * mt, 2 * mt + 1):
                    ff1_iter(1, i)

            out_proj(0, extra=_ff1_qc1)
            out_proj(1)

    nc.compile()
    return nc


def _get_program(with_bias: bool):
    key = ("nc", with_bias)
    if key not in _CACHED:
        _CACHED[key] = _build(with_bias)
    return _CACHED[key]


def kernel(x, context, ln_x_g, ln_x_b, ln_c_g, ln_c_b, Wq, Wkv, Wout, Wff1, Wff2):
    x = np.asarray(x, np.float32)
    context = np.asarray(context, np.float32)
    ln_x_g = np.asarray(ln_x_g, np.float32)
    ln_x_b = np.asarray(ln_x_b, np.float32)
    ln_c_g = np.asarray(ln_c_g, np.float32)
    ln_c_b = np.asarray(ln_c_b, np.float32)
    Wq = np.asarray(Wq, np.float32)
    Wkv = np.asarray(Wkv, np.float32)
    Wout = np.asarray(Wout, np.float32)
    Wff1 = np.asarray(Wff1, np.float32)
    Wff2 = np.asarray(Wff2, np.float32)

    # fold LN gains (and the attention scale) into the weights
    wq_eff = (ln_x_g[:, None] * Wq) * SCALE          # [1024, 1024]
    wkv_eff = ln_c_g[:, None] * Wkv                  # [1024, 128]
    # device kv layout: v at features 0:64, k at 64:128
    wkv_eff = np.concatenate([wkv_eff[:, DH:], wkv_eff[:, :DH]], axis=1)
    wff1_eff = ln_x_g[:, None] * Wff1                # [1024, 8192]
    with_bias = bool(np.any(ln_x_b != 0.0) or np.any(ln_c_b != 0.0))
    if with_bias:
        bq_eff = (ln_x_b @ Wq) * SCALE               # [1024]
        bkv_eff = ln_c_b @ Wkv                       # [128]
        bkv_eff = np.concatenate([bkv_eff[DH:], bkv_eff[:DH]])
        bff1_eff = ln_x_b @ Wff1                     # [8192]

    import ml_dtypes
    bf16 = ml_dtypes.bfloat16
    eye = np.eye(P, dtype=bf16)
    onesd = np.ones((P, 1), bf16)
    in_maps = []
    for c in range(8):
        s, t = c // 2, c % 2
        m = {
            "x": np.ascontiguousarray(x[s]),
            "ctx": np.ascontiguousarray(context[s]),
            "wq": np.ascontiguousarray(wq_eff[:, QF * t:QF * (t + 1)].astype(bf16)),
            "wkv": np.ascontiguousarray(wkv_eff.astype(bf16)),
            "wout": np.ascontiguousarray(Wout[QF * t:QF * (t + 1), :].astype(bf16)),
            "wff1": np.ascontiguousarray(np.concatenate(
                [wff1_eff[:, FFC * t:FFC * (t + 1)],
                 wff1_eff[:, 2 * FFC + FFC * t:2 * FFC + FFC * (t + 1)]],
                axis=1).astype(bf16)),
            "wff2": np.ascontiguousarray(Wff2[FFC * t:FFC * (t + 1), :].astype(bf16)),
            "eyer": eye,
            "onesd": onesd,
        }
        if with_bias:
            m["bq"] = np.ascontiguousarray(bq_eff[None, QF * t:QF * (t + 1)])
            m["bkv"] = np.ascontiguousarray(bkv_eff[None, :])
            m["bff1"] = np.ascontiguousarray(np.concatenate(
                [bff1_eff[None, FFC * t:FFC * (t + 1)],
                 bff1_eff[None, 2 * FFC + FFC * t:2 * FFC + FFC * (t + 1)]], axis=1))
        in_maps.append(m)

    nc = _get_program(with_bias)
    _CACHED["in_maps"] = in_maps
    res = bass_utils.run_bass_kernel_spmd(nc, in_maps, core_ids=list(range(8)))
    out = np.empty((B, NTOK, DIM), np.float32)
    for s in range(B):
        out[s] = (res.results[2 * s]["out"] + res.results[2 * s + 1]["out"]).T
    return out


# revision 15
# speedup vs baseline: 1.3410x; 1.1274x over previous
"""nn_CrossAttention Trainium2 Bass kernel.

Sharding (8 cores): data-parallel over batch (4 samples x 2 cores) with
2-way Megatron tensor parallelism inside each pair: core = (sample, half).
Each half owns 8 of 16 attention heads (Wq cols / Wout rows) and 2048 of
4096 ff_inner channels (Wff1 cols / Wff2 rows); the tiny shared-head Wkv is
replicated.  Per-core partial outputs (attn@Wout_half + ff@Wff2_half) are
summed pairwise on the host, which also owns the final transpose (the
device computes the output feature-major).

Device kernel (per core, identical SPMD program):
  - LayerNorm token-major via bn_stats (gains folded into the weights on
    the host), then PE-transpose to feature-major.
  - All matmuls in bf16 with 512-wide moving operands (fp32 PSUM accum).
  - Attention computed transposed (keys/queries feature-major, sim with
    context positions on partitions) so softmax sums fold into the
    attn@v matmul as a ones-column of the [v | 1] stationary operand.
  - FF1 (SwiGLU) interleaved with attention so TensorE hides ScalarE's
    exp() latency; out-projection accumulates the attention and FF paths
    into one PSUM group.
"""
import sys

if "/opt/trn_rl_repo" not in sys.path:
    sys.path.insert(0, "/opt/trn_rl_repo")

import numpy as np

import concourse.bass as bass  # noqa: F401  (bass must import before bacc)
import concourse.mybir as mybir
import concourse.tile as tile
from concourse import bacc, bass_utils

F32 = mybir.dt.float32
F32R = mybir.dt.float32r
BF16 = mybir.dt.bfloat16
AF = mybir.ActivationFunctionType
ALU = mybir.AluOpType

P = 128
B = 4           # batch
NTOK = 1024     # query tokens per sample
NCTX = 1024     # context tokens per sample
DIM = 1024
DH = 64         # head dim
HC = 8          # heads per core (16 total / 2-way TP)
QF = HC * DH    # 512 per-core q features
FFC = 2048      # per-core ff_inner channels
EPS = 1e-5
SCALE = DH ** -0.5

TT = NTOK // P   # 8 token tiles
KT = DIM // P    # 8 contraction tiles over dim
QC = NTOK // 512  # 2 moving-operand chunks of 512 tokens

_CACHED = {}


def _build(with_bias: bool):
    nc = bacc.Bacc("TRN2", target_bir_lowering=False, debug=False)

    x_d = nc.dram_tensor("x", [NTOK, DIM], F32, kind="ExternalInput").ap()
    c_d = nc.dram_tensor("ctx", [NCTX, DIM], F32, kind="ExternalInput").ap()
    wq_d = nc.dram_tensor("wq", [DIM, QF], BF16, kind="ExternalInput").ap()
    wkv_d = nc.dram_tensor("wkv", [DIM, 2 * DH], BF16, kind="ExternalInput").ap()
    wout_d = nc.dram_tensor("wout", [QF, DIM], BF16, kind="ExternalInput").ap()
    wff1_d = nc.dram_tensor("wff1", [DIM, 2 * FFC], BF16, kind="ExternalInput").ap()
    wff2_d = nc.dram_tensor("wff2", [FFC, DIM], BF16, kind="ExternalInput").ap()
    eyer_d = nc.dram_tensor("eyer", [P, P], BF16, kind="ExternalInput").ap()
    ones_d = nc.dram_tensor("onesd", [P, 1], BF16, kind="ExternalInput").ap()
    if with_bias:
        bq_d = nc.dram_tensor("bq", [1, QF], F32, kind="ExternalInput").ap()
        bkv_d = nc.dram_tensor("bkv", [1, 2 * DH], F32, kind="ExternalInput").ap()
        bff1_d = nc.dram_tensor("bff1", [1, 2 * FFC], F32, kind="ExternalInput").ap()
    out_d = nc.dram_tensor("out", [DIM, NTOK], F32, kind="ExternalOutput").ap()

    # dram views tiled for lhsT streaming: [p, ktile, cols]
    wq_v = wq_d.rearrange("(ko p) c -> p ko c", p=P)
    wkv_v = wkv_d.rearrange("(ko p) c -> p ko c", p=P)
    wout_v = wout_d.rearrange("(ko p) c -> p ko c", p=P)
    wff1_v = wff1_d.rearrange("(ko p) c -> p ko c", p=P)
    wff2_v = wff2_d.rearrange("(ko p) c -> p ko c", p=P)

    with tile.TileContext(nc) as tc:
        with (
            tc.tile_pool(name="consts", bufs=1) as consts,
            tc.tile_pool(name="ln", bufs=3) as lnp,
            tc.tile_pool(name="small", bufs=2) as smallp,
            tc.tile_pool(name="small1", bufs=1) as smallp1,
            tc.tile_pool(name="resid", bufs=1) as resid,
            tc.tile_pool(name="big", bufs=1) as bigp,
            tc.tile_pool(name="wst", bufs=4) as wst,
            tc.tile_pool(name="attn", bufs=2) as attnp,
            tc.tile_pool(name="pm", bufs=3, space="PSUM") as pmp,
            tc.tile_pool(name="po", bufs=2, space="PSUM") as pop,
            tc.tile_pool(name="pf", bufs=3, space="PSUM") as pfp,
        ):
            identr = consts.tile([P, P], BF16)
            nc.sync.dma_start(identr[:], eyer_d[:])
            eps_t = consts.tile([P, 1], F32)
            nc.vector.memset(eps_t[:], EPS)
            if with_bias:
                bq_t = consts.tile([P, QF // P], F32)
                nc.sync.dma_start(bq_t[:], bq_d.rearrange("o (fo p) -> p (o fo)", p=P))
                bkv_t = consts.tile([P, 1], F32)
                nc.sync.dma_start(bkv_t[:], bkv_d.rearrange("o (fo p) -> p (o fo)", p=P))
                bff1_t = consts.tile([P, (2 * FFC) // P], F32)
                nc.sync.dma_start(
                    bff1_t[:], bff1_d.rearrange("o (fo p) -> p (o fo)", p=P)
                )

            # persistent activations
            xn_F = resid.tile([P, KT, NTOK], BF16)      # normalized x, feature-major
            qT = resid.tile([P, QF // P, NTOK], BF16)   # queries, feature-major
            kv_sb = resid.tile([P, NCTX], BF16)         # rows 0:64 v, 64:128 k (feature-major)
            kdup = resid.tile([P, NCTX], BF16)          # rows 0:64 = copy of k
            v_aug = resid.tile([P, NCTX // P, DH + 1], BF16)  # [j-in-tile, jt, v|1]
            attn_outT = resid.tile([P, QF // P, NTOK], BF16)  # attention out, feature-major

            def layernorm_iter(src_dram, dst_fmajor, tt):
                if True:
                    xt = wst.tile([P, DIM], F32, tag="wbig", name="xt")
                    nc.sync.dma_start(xt[:], src_dram[tt * P:(tt + 1) * P, :])
                    st = lnp.tile([P, 2, nc.vector.BN_STATS_DIM], F32, tag="lnst")
                    xv = xt.rearrange("p (s f) -> p s f", s=2)
                    nc.vector.bn_stats(st[:, 0, :], xv[:, 0, :])
                    nc.vector.bn_stats(st[:, 1, :], xv[:, 1, :])
                    mv = lnp.tile([P, nc.vector.BN_AGGR_DIM], F32, tag="lnmv")
                    nc.vector.bn_aggr(mv[:], st[:])
                    # rstd = 1/sqrt(var + eps)
                    rstd = lnp.tile([P, 1], F32, tag="lnrs")
                    nc.scalar.activation(
                        out=rstd[:], in_=mv[:, 1:2], func=AF.Sqrt, bias=eps_t[:]
                    )
                    nc.vector.reciprocal(rstd[:], rstd[:])
                    xh = lnp.tile([P, DIM], BF16, tag="lnh")
                    nc.vector.tensor_scalar(
                        out=xh[:], in0=xt[:], scalar1=mv[:, 0:1], scalar2=rstd[:],
                        op0=ALU.subtract, op1=ALU.mult,
                    )
                    for dt_ in range(KT):
                        pt = pmp.tile([P, 512], BF16, tag="pm", name="pt")
                        nc.tensor.transpose(
                            pt[:, 0:P], xh[:, dt_ * P:(dt_ + 1) * P], identr[:]
                        )
                        if dt_ % 2 == 0:
                            nc.vector.tensor_copy(
                                dst_fmajor[:, dt_, tt * P:(tt + 1) * P], pt[:, 0:P]
                            )
                        else:
                            nc.scalar.activation(
                                out=dst_fmajor[:, dt_, tt * P:(tt + 1) * P],
                                in_=pt[:, 0:P], func=AF.Copy,
                            )

            # ---- phase A: layernorm(x) ----
            for tt in range(TT):
                layernorm_iter(x_d, xn_F, tt)
            cn_F = bigp.tile([P, KT, NCTX], BF16, tag="bigc", name="cn_F")

            # ---- phase D: q = xn @ Wq, interleaved with layernorm(ctx) ----
            for ft in range(QF // P):
                layernorm_iter(c_d, cn_F, 2 * ft)
                layernorm_iter(c_d, cn_F, 2 * ft + 1)
                wq_t = wst.tile([P, KT, P], BF16, tag="wpair", name="wq_t")
                nc.sync.dma_start(wq_t[:], wq_v[:, :, ft * P:(ft + 1) * P])
                for qc in range(QC):
                    pq = pmp.tile([P, 512], F32, tag="pm")
                    for k in range(KT):
                        nc.tensor.matmul(
                            pq[:], wq_t[:, k, :],
                            xn_F[:, k, qc * 512:(qc + 1) * 512],
                            start=(k == 0), stop=(k == KT - 1),
                        )
                    if with_bias:
                        nc.vector.tensor_scalar_add(
                            out=qT[:, ft, qc * 512:(qc + 1) * 512],
                            in0=pq[:], scalar1=bq_t[:, ft:ft + 1],
                        )
                    else:
                        nc.vector.tensor_copy(
                            qT[:, ft, qc * 512:(qc + 1) * 512], pq[:]
                        )

            # ---- phase C: kv = cn @ Wkv (feature-major out) ----
            wkv_t = wst.tile([P, KT, 2 * DH], BF16, tag="wpair", name="wkv_t")
            nc.sync.dma_start(wkv_t[:], wkv_v[:])
            for jc in range(NCTX // 512):
                pkv = pmp.tile([P, 512], F32, tag="pm")
                for k in range(KT):
                    nc.tensor.matmul(
                        pkv[0:2 * DH, :], wkv_t[:, k, :],
                        cn_F[:, k, jc * 512:(jc + 1) * 512],
                        start=(k == 0), stop=(k == KT - 1),
                    )
                if with_bias:
                    nc.vector.tensor_scalar_add(
                        out=kv_sb[:, jc * 512:(jc + 1) * 512],
                        in0=pkv[0:2 * DH, :], scalar1=bkv_t[:],
                    )
                else:
                    nc.vector.tensor_copy(
                        kv_sb[:, jc * 512:(jc + 1) * 512], pkv[0:2 * DH, :]
                    )
            # k lives at partitions 64:128 (for the odd-head sim matmuls, whose
            # queries sit at partitions 64:128); duplicate it at 0:64 for the
            # even heads.
            nc.sync.dma_start(kdup[0:DH, :], kv_sb[DH:2 * DH, :])
            # v (partitions 0:64) transposed to token-major with a ones column
            for jt in range(NCTX // P):
                pv = pmp.tile([P, 512], BF16, tag="pm")
                nc.tensor.transpose(
                    pv[:, 0:DH], kv_sb[0:DH, jt * P:(jt + 1) * P],
                    identr[0:DH, 0:DH],
                )
                nc.vector.tensor_copy(v_aug[:, jt, 0:DH], pv[:, 0:DH])
            nc.sync.dma_start(
                v_aug[:, :, DH:DH + 1],
                bass.AP(tensor=ones_d.tensor, offset=0,
                        ap=[list(ones_d.ap[0]), [0, NCTX // P], list(ones_d.ap[1])]),
            )

            # ---- phases E+F: attention (pairs of heads) interleaved with FF1 ----
            ff_sc = [None, None]  # per-qc silu(gate)*val, feature-major [128, 16, 512]

            def ff1_iter(qc, i):
                """One val/gate column pair (128 wide) of the SwiGLU FF."""
                wv_t = wst.tile([P, KT, P], BF16, tag="wpair", name="wv_t")
                nc.sync.dma_start(wv_t[:], wff1_v[:, :, i * P:(i + 1) * P])
                wg_t = wst.tile([P, KT, P], BF16, tag="wpair", name="wg_t")
                nc.sync.dma_start(wg_t[:], wff1_v[:, :, FFC + i * P:FFC + (i + 1) * P])
                pv_ = pfp.tile([P, 512], F32, tag="pf")
                pg_ = pfp.tile([P, 512], F32, tag="pf")
                for k in range(KT):
                    nc.tensor.matmul(
                        pv_[:], wv_t[:, k, :], xn_F[:, k, qc * 512:(qc + 1) * 512],
                        start=(k == 0), stop=(k == KT - 1),
                    )
                for k in range(KT):
                    nc.tensor.matmul(
                        pg_[:], wg_t[:, k, :], xn_F[:, k, qc * 512:(qc + 1) * 512],
                        start=(k == 0), stop=(k == KT - 1),
                    )
                if with_bias:
                    nc.vector.tensor_scalar_add(
                        out=pv_[:], in0=pv_[:], scalar1=bff1_t[:, i:i + 1]
                    )
                    nc.vector.tensor_scalar_add(
                        out=pg_[:], in0=pg_[:], scalar1=bff1_t[:, FFC // P + i:FFC // P + i + 1]
                    )
                sg = smallp.tile([P, 512], F32, tag="silu")
                nc.scalar.activation(out=sg[:], in_=pg_[:], func=AF.Sigmoid)
                nc.vector.tensor_tensor(sg[:], pg_[:], sg[:], ALU.mult)
                nc.vector.tensor_tensor(
                    ff_sc[qc][:, i, :], pv_[:], sg[:], ALU.mult
                )

            def attn_pair(ft, qc):
                """Heads (2ft, 2ft+1) for one 512-token chunk."""
                expT = attnp.tile([P, NCTX // P, 2, 512], BF16, tag="expT")
                qsl = [
                    qT[0:DH, ft, qc * 512:(qc + 1) * 512],
                    qT[DH:2 * DH, ft, qc * 512:(qc + 1) * 512],
                ]
                for jt in range(NCTX // P):
                    ps0 = pmp.tile([P, 512], F32, tag="pm")
                    ps1 = pmp.tile([P, 512], F32, tag="pm")
                    nc.tensor.matmul(
                        ps0[:], kdup[0:DH, jt * P:(jt + 1) * P], qsl[0],
                        start=True, stop=True,
                    )
                    nc.tensor.matmul(
                        ps1[:], kv_sb[DH:2 * DH, jt * P:(jt + 1) * P], qsl[1],
                        start=True, stop=True,
                    )
                    nc.scalar.activation(out=expT[:, jt, 0, :], in_=ps0[:], func=AF.Exp)
                    nc.scalar.activation(out=expT[:, jt, 1, :], in_=ps1[:], func=AF.Exp)
                po = [pop.tile([P, 512], F32, tag="po", name=f"po{e}") for e in range(2)]
                for jt in range(NCTX // P):
                    for e in range(2):
                        nc.tensor.matmul(
                            po[e][0:DH + 1, :], v_aug[:, jt, :], expT[:, jt, e, :],
                            start=(jt == 0), stop=(jt == NCTX // P - 1),
                        )
                for e in range(2):
                    rec = smallp1.tile([P, 512], F32, tag="rec")
                    # move the sums row (psum partition 64) to partition 0
                    nc.vector.tensor_copy(rec[DH:DH + 1, :], po[e][DH:DH + 1, :])
                    nc.sync.dma_start(rec[0:1, :], rec[DH:DH + 1, :])
                    nc.vector.reciprocal_approx_fast(out=rec[0:1, :], in_=rec[0:1, :])
                    rb = smallp1.tile([DH, 512], F32, tag="rb")
                    nc.gpsimd.partition_broadcast(rb[:], rec[0:1, :])
                    if e == 0:
                        nc.vector.tensor_tensor(
                            attn_outT[0:DH, ft, qc * 512:(qc + 1) * 512],
                            po[e][0:DH, :], rb[:], ALU.mult,
                        )
                    else:
                        stg = smallp1.tile([DH, 512], BF16, tag="stg")
                        nc.vector.tensor_tensor(stg[:], po[e][0:DH, :], rb[:], ALU.mult)
                        nc.sync.dma_start(
                            attn_outT[DH:2 * DH, ft, qc * 512:(qc + 1) * 512], stg[:]
                        )

            ff_sc[0] = bigp.tile([P, FFC // P, 512], BF16, tag="big", name="ff_sc0")
            ff_it = iter(range(FFC // P))
            for ft in range(QF // P):
                for qc in range(QC):
                    attn_pair(ft, qc)
                    for _ in range(2):
                        i = next(ff_it, None)
                        if i is not None:
                            ff1_iter(0, i)
            for i in ff_it:
                ff1_iter(0, i)

            # ---- phase G: out = attn_outT' Wout + ff' Wff2, per 512-token chunk ----
            def out_proj(qc, extra=None):
                for mt in range(DIM // P):
                    if extra is not None:
                        extra(mt)
                    wo_t = wst.tile([P, QF // P, P], BF16, tag="wpair", name="wo_t")
                    nc.sync.dma_start(wo_t[:], wout_v[:, :, mt * P:(mt + 1) * P])
                    wf2_t = wst.tile([P, FFC // P, P], BF16, tag="wbig", name="wf2_t")
                    nc.sync.dma_start(wf2_t[:], wff2_v[:, :, mt * P:(mt + 1) * P])
                    pout = pmp.tile([P, 512], F32, tag="pm")
                    nkt = QF // P + FFC // P
                    for k in range(QF // P):
                        nc.tensor.matmul(
                            pout[:], wo_t[:, k, :],
                            attn_outT[:, k, qc * 512:(qc + 1) * 512],
                            start=(k == 0), stop=False,
                        )
                    for k in range(FFC // P):
                        nc.tensor.matmul(
                            pout[:], wf2_t[:, k, :], ff_sc[qc][:, k, :],
                            start=False, stop=(k == FFC // P - 1),
                        )
                    ot = smallp.tile([P, 512], F32, tag="ot")
                    nc.vector.tensor_copy(ot[:], pout[:])
                    nc.sync.dma_start(
                        out_d[mt * P:(mt + 1) * P, qc * 512:(qc + 1) * 512], ot[:]
                    )

            ff_sc[1] = bigp.tile([P, FFC // P, 512], BF16, tag="big2", name="ff_sc1")

            def _ff1_qc1(mt):
                for i in (2 * mt, 2 * mt + 1):
                    ff1_iter(1, i)

            out_proj(0, extra=_ff1_qc1)
            out_proj(1)

    nc.compile()
    return nc


def _get_program(with_bias: bool):
    key = ("nc", with_bias)
    if key not in _CACHED:
        _CACHED[key] = _build(with_bias)
    return _CACHED[key]


def kernel(x, context, ln_x_g, ln_x_b, ln_c_g, ln_c_b, Wq, Wkv, Wout, Wff1, Wff2):
    x = np.asarray(x, np.float32)
    context = np.asarray(context, np.float32)
    ln_x_g = np.asarray(ln_x_g, np.float32)
    ln_x_b = np.asarray(ln_x_b, np.float32)
    ln_c_g = np.asarray(ln_c_g, np.float32)
    ln_c_b = np.asarray(ln_c_b, np.float32)
    Wq = np.asarray(Wq, np.float32)
    Wkv = np.asarray(Wkv, np.float32)
    Wout = np.asarray(Wout, np.float32)
    Wff1 = np.asarray(Wff1, np.float32)
    Wff2 = np.asarray(Wff2, np.float32)

    # fold LN gains (and the attention scale) into the weights
    wq_eff = (ln_x_g[:, None] * Wq) * SCALE          # [1024, 1024]
    wkv_eff = ln_c_g[:, None] * Wkv                  # [1024, 128]
    # device kv layout: v at features 0:64, k at 64:128
    wkv_eff = np.concatenate([wkv_eff[:, DH:], wkv_eff[:, :DH]], axis=1)
    wff1_eff = ln_x_g[:, None] * Wff1                # [1024, 8192]
    with_bias = bool(np.any(ln_x_b != 0.0) or np.any(ln_c_b != 0.0))
    if with_bias:
        bq_eff = (ln_x_b @ Wq) * SCALE               # [1024]
        bkv_eff = ln_c_b @ Wkv                       # [128]
        bkv_eff = np.concatenate([bkv_eff[DH:], bkv_eff[:DH]])
        bff1_eff = ln_x_b @ Wff1                     # [8192]

    import ml_dtypes
    bf16 = ml_dtypes.bfloat16
    eye = np.eye(P, dtype=bf16)
    onesd = np.ones((P, 1), bf16)
    in_maps = []
    for c in range(8):
        s, t = c // 2, c % 2
        m = {
            "x": np.ascontiguousarray(x[s]),
            "ctx": np.ascontiguousarray(context[s]),
            "wq": np.ascontiguousarray(wq_eff[:, QF * t:QF * (t + 1)].astype(bf16)),
            "wkv": np.ascontiguousarray(wkv_eff.astype(bf16)),
            "wout": np.ascontiguousarray(Wout[QF * t:QF * (t + 1), :].astype(bf16)),
            "wff1": np.ascontiguousarray(np.concatenate(
                [wff1_eff[:, FFC * t:FFC * (t + 1)],
                 wff1_eff[:, 2 * FFC + FFC * t:2 * FFC + FFC * (t + 1)]],
                axis=1).astype(bf16)),
            "wff2": np.ascontiguousarray(Wff2[FFC * t:FFC * (t + 1), :].astype(bf16)),
            "eyer": eye,
            "onesd": onesd,
        }
        if with_bias:
            m["bq"] = np.ascontiguousarray(bq_eff[None, QF * t:QF * (t + 1)])
            m["bkv"] = np.ascontiguousarray(bkv_eff[None, :])
            m["bff1"] = np.ascontiguousarray(np.concatenate(
                [bff1_eff[None, FFC * t:FFC * (t + 1)],
                 bff1_eff[None, 2 * FFC + FFC * t:2 * FFC + FFC * (t + 1)]], axis=1))
        in_maps.append(m)

    nc = _get_program(with_bias)
    _CACHED["in_maps"] = in_maps
    res = bass_utils.run_bass_kernel_spmd(nc, in_maps, core_ids=list(range(8)))
    out = np.empty((B, NTOK, DIM), np.float32)
    for s in range(B):
        out[s] = (res.results[2 * s]["out"] + res.results[2 * s + 1]["out"]).T
    return out
